# revision 20
# baseline (speedup 1.0000x reference)
"""Trainium2 Bass kernel for EquidistantDiscreteContinuousConv3d.

Math: out = conv3d(x, einsum('ogk,kzyx->ogzyx', weight, psi_local), stride 2,
pad 2) + bias, with x [2,8,128,128,128] -> out [2,16,64,64,64].

The dense 5^3 kernel only has taps within Euclidean radius 2 (33 of 125
offsets are nonzero). Sharding: 8 cores = batch(2) x z-groups(4); each core
computes out[b, :, 16g:16g+16] from an overlapping, zero-padded input slab.
No collectives — halos materialize as overlapping host-side slices.

Device mapping: the tensor engine contracts K = (z_local(16) x ic(8)) = 128
partitions, with M = (oz_sub x oc(16)) packed into a block-banded weight
matrix (band encodes the 5 dz taps, zeros elsewhere), looped over the 13
(dy, dx) stencil taps that accumulate in PSUM. rhs slices come from a
phase-decomposed (even/odd y and x, de-interleaved so the innermost 64
x-positions are contiguous) view of the input tile. Inputs arrive as 12
y-halved sub-units, each as two overlapping half-DMAs, so the first matmul
starts after ~1 MB of DMA and transfers stay at full bandwidth (at most two
in flight, same-kind halves never concurrent so per-kind semaphore counts
are completion-exact).

Raw Bacc pipeline per core (static, fully unrolled; no TileContext):
  SP  : 24 paced input half-DMAs (xt slot = unit%5), then end-of-run sem clear
  ACT : wtile DMA, then 12 half-stage output DMAs
  PE  : 24 groups x 13 banded matmuls accumulating in psum bank g%8
  DVE : 24 psum->stage copies (stage slot = s%2)
"""

import os

import ml_dtypes
import numpy as np

BF16 = ml_dtypes.bfloat16

IC, OC = 8, 16
TAPS_XY = [
    (dy, dx) for dy in range(-2, 3) for dx in range(-2, 3) if dy * dy + dx * dx <= 4
]  # 13 taps
OZ_PER = (6, 6, 4)
SUB_FREE = 36 * 132  # y-half sub-unit free size: (yo 18, yp 2, px 2, xe 66)
N_CORES = 8

_MODULE = None
LAST_RESULT = None  # BassKernelResults of the most recent run (for test harness)


def _build_module():
    from contextlib import ExitStack

    import concourse.bacc as bacc
    import concourse.mybir as mybir

    f32 = mybir.dt.float32
    bf16 = mybir.dt.bfloat16

    nc = bacc.Bacc()
    x_in = nc.dram_tensor("xc", [12, 128, SUB_FREE], bf16, kind="ExternalInput")
    w_in = nc.dram_tensor("wc", [128, 13 * 128], bf16, kind="ExternalInput")
    out = nc.dram_tensor("out", [16, 16, 64, 64], f32, kind="ExternalOutput")

    NG = 24  # groups: g = (((c*2)+h)*2+q)*2+tt
    NSLOT = 5
    ROW = 2 * 2 * 66  # one yo row = (yp, px, xe) block of 264 elements

    def gdec(g):
        c, r = divmod(g, 8)
        h, r = divmod(r, 4)
        q, tt = divmod(r, 2)
        return c, h, q, tt

    with ExitStack() as ctx:
        wsem = ctx.enter_context(nc.semaphore("wsem"))
        xsA = [ctx.enter_context(nc.semaphore(f"xsemA{i}")) for i in range(2)]
        xsB = [ctx.enter_context(nc.semaphore(f"xsemB{i}")) for i in range(2)]
        pesem = ctx.enter_context(nc.semaphore("pesem"))
        dvsem = ctx.enter_context(nc.semaphore("dvsem"))
        osem = ctx.enter_context(nc.semaphore("osem"))
        wtile = ctx.enter_context(nc.sbuf_tensor("wtile", [128, 13 * 128], bf16))
        xts = [
            ctx.enter_context(nc.sbuf_tensor(f"xt{i}", [128, SUB_FREE], bf16))
            for i in range(NSLOT)
        ]
        stgs = [
            ctx.enter_context(nc.sbuf_tensor(f"stg{i}", [128, 4 * 512], f32))
            for i in range(2)
        ]
        pss = [
            ctx.enter_context(nc.psum_tensor(f"ps{i}", [128, 512], f32))
            for i in range(8)
        ]
        x5s = [
            t[:].rearrange("p (a b d c) -> p a b d c", a=18, b=2, d=2, c=66)
            for t in xts
        ]

        with nc.Block() as block:

            @block.sync
            def _(sp):
                # half A = yo [0,10) (enough for the tt=0 group); A halves all
                # ride the SP queue, B halves the ACT queue. Each queue
                # alternates between two sems so receipts overlap transfers
                # while same-sem counts stay completion-exact.
                for i in range(12):
                    if i == 1:
                        sp.wait_ge(xsA[0], 16)  # let A0 land at full BW
                    elif i >= 2:
                        sp.wait_ge(xsA[i % 2], 16 * (i // 2))
                    if i >= NSLOT:
                        sp.wait_ge(pesem, 2 * (i - NSLOT) + 2)  # slot free
                    sp.dma_start(
                        out=xts[i % NSLOT][:, 0 : 10 * ROW],
                        in_=x_in[i, :, 0 : 10 * ROW],
                    ).then_inc(xsA[i % 2], 16)
                # re-execution safety: clear sems once everything is done
                sp.wait_ge(osem, 16 * 12)  # all 12 out DMAs done
                for sem in (wsem, xsA[0], xsA[1], xsB[0], xsB[1], pesem, dvsem, osem):
                    sp.sem_clear(sem)

            @block.scalar
            def _(act):
                act.dma_start(out=wtile[:], in_=w_in[:]).then_inc(wsem, 16)

                def bdma(i):
                    if i == 0:
                        pass  # B0 shares the ramp with A0/wtile
                    elif i == 1:
                        act.wait_ge(xsB[0], 16)
                    else:
                        act.wait_ge(xsB[i % 2], 16 * (i // 2))
                    if i >= NSLOT:
                        act.wait_ge(pesem, 2 * (i - NSLOT) + 2)
                    act.dma_start(
                        out=xts[i % NSLOT][:, 8 * ROW : 18 * ROW],
                        in_=x_in[i, :, 8 * ROW : 18 * ROW],
                    ).then_inc(xsB[i % 2], 16)

                def odma(s, uh):
                    c, h = divmod(s, 2)
                    M = OZ_PER[c] * 16
                    act.wait_ge(dvsem, 4 * s + 2 * (uh + 1))
                    dst = out[
                        6 * c : 6 * c + OZ_PER[c],
                        :,
                        32 * h + 16 * uh : 32 * h + 16 * uh + 16,
                        :,
                    ].rearrange("a b c d -> (a b) (c d)")
                    act.dma_start(
                        out=dst, in_=stgs[s % 2][:M, 1024 * uh : 1024 * uh + 1024]
                    ).then_inc(osem, 16)

                # interleave B-half inputs with output stages so neither
                # starves: B(ui) is needed well after out(s) waits clear
                for i in range(5):
                    bdma(i)
                k = 5
                for s in range(6):
                    for uh in range(2):
                        odma(s, uh)
                        if k < 12:
                            bdma(k)
                            k += 1

            @block.tensor
            def _(pe):
                # HAM warm-up during the DMA ramp: throwaway matmuls (inputs
                # may be mid-DMA garbage) into psum bank 7, discarded by that
                # bank's first real start=True matmul
                for _ in range(9):
                    pe.matmul(
                        pss[7][:], wtile[:, 0:128], wtile[:, 0:512],
                        start=True, stop=True,
                    )
                pe.wait_ge(wsem, 16)
                for g in range(NG):
                    c, h, q, tt = gdec(g)
                    i = g // 2
                    pe.wait_ge(xsA[i % 2], 16 * (i // 2 + 1))
                    if tt == 1:
                        pe.wait_ge(xsB[i % 2], 16 * (i // 2 + 1))
                    if g >= 8:
                        pe.wait_ge(dvsem, g - 7)  # psum bank g%8 evacuated
                    x5 = x5s[i % NSLOT]
                    ps = pss[g % 8]
                    for j, (dy, dx) in enumerate(TAPS_XY):
                        jy, py = divmod(dy + 2, 2)
                        jx, px = divmod(dx + 2, 2)
                        a0 = 8 * tt + jy
                        rhs = x5[
                            :, a0 : a0 + 8, py : py + 1, px : px + 1, jx : jx + 64
                        ]
                        mm = pe.matmul(
                            ps[:],
                            wtile[:, j * 128 : (j + 1) * 128],
                            rhs,
                            start=(j == 0),
                            stop=(j == len(TAPS_XY) - 1),
                        )
                        if j == len(TAPS_XY) - 1:
                            mm.then_inc(pesem, 1)

            @block.vector
            def _(dve):
                for g in range(NG):
                    s = g // 4
                    t = g % 4
                    M = OZ_PER[g // 8] * 16
                    if t == 0 and s >= 2:
                        dve.wait_ge(osem, 32 * (s - 1))  # stage slot s%2 free
                    dve.wait_ge(pesem, g + 1)
                    dve.tensor_copy(
                        out=stgs[s % 2][:M, t * 512 : (t + 1) * 512],
                        in_=pss[g % 8][:M],
                    ).then_inc(dvsem, 1)

    nc.compile()
    return nc


def _get_module():
    global _MODULE
    if _MODULE is None:
        _MODULE = _build_module()
    return _MODULE


def _band_weights(w5):
    """wc[k=(z*8+ic), j*128 + ozs*16 + oc] block-banded weights (ozs 6,7 pad)."""
    wc = np.zeros((128, 13, 8, 16), np.float32)
    for j, (dy, dx) in enumerate(TAPS_XY):
        for dzi in range(5):
            dz = dzi - 2
            if dz * dz + dy * dy + dx * dx > 4:
                continue
            blk = w5[:, :, dzi, dy + 2, dx + 2].T  # [ic, oc]
            for ozs in range(6):
                z = 2 * ozs + dzi
                wc[z * 8 : (z + 1) * 8, j, ozs, :] = blk
    return np.ascontiguousarray(wc.reshape(128, 13 * 128))


def _shard_core_input(x, b, gz):
    """Per-core padded input as 12 y-halved sub-units [128, 36*132]."""
    xp = np.zeros((IC, 40, 132, 132), BF16)
    z_lo = 32 * gz - 2
    src_lo, src_hi = max(0, z_lo), min(128, z_lo + 40)
    xp[:, src_lo - z_lo : src_hi - z_lo, 2:130, 2:130] = x[b, :, src_lo:src_hi, :, :]
    units = np.empty((12, 128, SUB_FREE), BF16)
    for c in range(3):
        for h in range(2):
            for q in range(2):
                u = xp[:, 12 * c : 12 * c + 16, 64 * h + 32 * q : 64 * h + 32 * q + 36, :]
                # de-interleave phases: free = (yo 18, yp 2, px 2, xe 66)
                u = u.reshape(IC, 16, 36, 66, 2).transpose(0, 1, 2, 4, 3)
                u = u.reshape(IC, 16, 18, 2, 2, 66)
                units[c * 4 + h * 2 + q] = u.transpose(1, 0, 2, 3, 4, 5).reshape(
                    128, SUB_FREE
                )
    return units


def kernel(x, weight, bias, psi_local):
    global LAST_RESULT
    from concourse.bass_utils import run_bass_kernel_spmd

    x = np.asarray(x, np.float32)
    weight = np.asarray(weight, np.float32)
    bias = np.asarray(bias, np.float32)
    psi_local = np.asarray(psi_local, np.float32)

    w5 = np.einsum("ogk,kzyx->ogzyx", weight, psi_local).astype(np.float32)
    wc = _band_weights(w5).astype(BF16)

    in_maps = []
    for core in range(N_CORES):
        b, gz = divmod(core, 4)
        in_maps.append({"xc": _shard_core_input(x, b, gz), "wc": wc})

    nc = _get_module()
    trace = bool(int(os.environ.get("KERNEL_TRACE", "0")))
    res = run_bass_kernel_spmd(
        nc, in_maps, core_ids=list(range(N_CORES)), trace=trace
    )
    LAST_RESULT = res

    out = np.empty((2, OC, 64, 64, 64), np.float32)
    for core in range(N_CORES):
        b, gz = divmod(core, 4)
        out[b, :, 16 * gz : 16 * gz + 16] = res.results[core]["out"].transpose(
            1, 0, 2, 3
        )
    out += bias[None, :, None, None, None]
    return out


# revision 21
# speedup vs baseline: 1.0122x; 1.0122x over previous
"""Trainium2 Bass kernel for EquidistantDiscreteContinuousConv3d.

Math: out = conv3d(x, einsum('ogk,kzyx->ogzyx', weight, psi_local), stride 2,
pad 2) + bias, with x [2,8,128,128,128] -> out [2,16,64,64,64].

The dense 5^3 kernel only has taps within Euclidean radius 2 (33 of 125
offsets are nonzero). Sharding: 8 cores = batch(2) x z-groups(4); each core
computes out[b, :, 16g:16g+16] from an overlapping, zero-padded input slab.
No collectives — halos materialize as overlapping host-side slices.

Device mapping: the tensor engine contracts K = (z_local(16) x ic(8)) = 128
partitions, with M = (oz_sub x oc(16)) packed into a block-banded weight
matrix (band encodes the 5 dz taps, zeros elsewhere), looped over the 13
(dy, dx) stencil taps that accumulate in PSUM. rhs slices come from a
phase-decomposed (even/odd y and x, de-interleaved so the innermost 64
x-positions are contiguous) view of the input tile. Inputs arrive as 12
y-halved sub-units, each as two overlapping half-DMAs, so the first matmul
starts after ~1 MB of DMA and transfers stay at full bandwidth (at most two
in flight, same-kind halves never concurrent so per-kind semaphore counts
are completion-exact).

Raw Bacc pipeline per core (static, fully unrolled; no TileContext):
  SP  : 24 paced input half-DMAs (xt slot = unit%5), then end-of-run sem clear
  ACT : wtile DMA, then 12 half-stage output DMAs
  PE  : 24 groups x 13 banded matmuls accumulating in psum bank g%8
  DVE : 24 psum->stage copies (stage slot = s%2)
"""

import os

import ml_dtypes
import numpy as np

BF16 = ml_dtypes.bfloat16

IC, OC = 8, 16
TAPS_XY = [
    (dy, dx) for dy in range(-2, 3) for dx in range(-2, 3) if dy * dy + dx * dx <= 4
]  # 13 taps
OZ_PER = (6, 6, 4)
SUB_FREE = 36 * 132  # y-half sub-unit free size: (yo 18, yp 2, px 2, xe 66)
N_CORES = 8

_MODULE = None
LAST_RESULT = None  # BassKernelResults of the most recent run (for test harness)


def _build_module():
    from contextlib import ExitStack

    import concourse.bacc as bacc
    import concourse.mybir as mybir

    f32 = mybir.dt.float32
    bf16 = mybir.dt.bfloat16

    nc = bacc.Bacc()
    x_in = nc.dram_tensor("xc", [12, 128, SUB_FREE], bf16, kind="ExternalInput")
    w_in = nc.dram_tensor("wc", [128, 13 * 128], bf16, kind="ExternalInput")
    out = nc.dram_tensor("out", [16, 16, 64, 64], f32, kind="ExternalOutput")

    NG = 24  # groups: g = (((c*2)+h)*2+q)*2+tt
    NSLOT = 5
    ROW = 2 * 2 * 66  # one yo row = (yp, px, xe) block of 264 elements

    def gdec(g):
        c, r = divmod(g, 8)
        h, r = divmod(r, 4)
        q, tt = divmod(r, 2)
        return c, h, q, tt

    with ExitStack() as ctx:
        wsem = ctx.enter_context(nc.semaphore("wsem"))
        xsA = [ctx.enter_context(nc.semaphore(f"xsemA{i}")) for i in range(2)]
        xsB = [ctx.enter_context(nc.semaphore(f"xsemB{i}")) for i in range(2)]
        pesem = ctx.enter_context(nc.semaphore("pesem"))
        dvsem = ctx.enter_context(nc.semaphore("dvsem"))
        osem = ctx.enter_context(nc.semaphore("osem"))
        wtile = ctx.enter_context(nc.sbuf_tensor("wtile", [128, 13 * 128], bf16))
        xts = [
            ctx.enter_context(nc.sbuf_tensor(f"xt{i}", [128, SUB_FREE], bf16))
            for i in range(NSLOT)
        ]
        stgs = [
            ctx.enter_context(nc.sbuf_tensor(f"stg{i}", [128, 4 * 512], f32))
            for i in range(2)
        ]
        pss = [
            ctx.enter_context(nc.psum_tensor(f"ps{i}", [128, 512], f32))
            for i in range(8)
        ]
        x5s = [
            t[:].rearrange("p (a b d c) -> p a b d c", a=18, b=2, d=2, c=66)
            for t in xts
        ]

        with nc.Block() as block:

            @block.sync
            def _(sp):
                # half A = yo [0,10) (enough for the tt=0 group); A halves all
                # ride the SP queue, B halves the ACT queue. Each queue
                # alternates between two sems so receipts overlap transfers
                # while same-sem counts stay completion-exact.
                for i in range(12):
                    if i == 1:
                        sp.wait_ge(xsA[0], 16)  # let A0 land at full BW
                    elif i >= 2:
                        sp.wait_ge(xsA[i % 2], 16 * (i // 2))
                    if i >= NSLOT:
                        sp.wait_ge(pesem, 2 * (i - NSLOT) + 2)  # slot free
                    sp.dma_start(
                        out=xts[i % NSLOT][:, 0 : 10 * ROW],
                        in_=x_in[i, :, 0 : 10 * ROW],
                    ).then_inc(xsA[i % 2], 16)
                # re-execution safety: clear sems once everything is done
                sp.wait_ge(osem, 16 * 12)  # all 12 out DMAs done
                for sem in (wsem, xsA[0], xsA[1], xsB[0], xsB[1], pesem, dvsem, osem):
                    sp.sem_clear(sem)

            @block.scalar
            def _(act):
                act.dma_start(out=wtile[:], in_=w_in[:]).then_inc(wsem, 16)

                def bdma(i):
                    if i == 0:
                        pass  # B0 shares the ramp with A0/wtile
                    elif i == 1:
                        act.wait_ge(xsB[0], 16)
                    else:
                        act.wait_ge(xsB[i % 2], 16 * (i // 2))
                    if i >= NSLOT:
                        act.wait_ge(pesem, 2 * (i - NSLOT) + 2)
                    act.dma_start(
                        out=xts[i % NSLOT][:, 8 * ROW : 18 * ROW],
                        in_=x_in[i, :, 8 * ROW : 18 * ROW],
                    ).then_inc(xsB[i % 2], 16)

                def odma(s, uh):
                    c, h = divmod(s, 2)
                    M = OZ_PER[c] * 16
                    act.wait_ge(dvsem, 4 * s + 2 * (uh + 1))
                    dst = out[
                        6 * c : 6 * c + OZ_PER[c],
                        :,
                        32 * h + 16 * uh : 32 * h + 16 * uh + 16,
                        :,
                    ].rearrange("a b c d -> (a b) (c d)")
                    act.dma_start(
                        out=dst, in_=stgs[s % 2][:M, 1024 * uh : 1024 * uh + 1024]
                    ).then_inc(osem, 16)

                # interleave B-half inputs with output stages so neither
                # starves: B(ui) is needed well after out(s) waits clear
                for i in range(5):
                    bdma(i)
                k = 5
                for s in range(6):
                    for uh in range(2):
                        odma(s, uh)
                        if k < 12:
                            bdma(k)
                            k += 1

            @block.tensor
            def _(pe):
                pe.wait_ge(wsem, 16)
                for g in range(NG):
                    c, h, q, tt = gdec(g)
                    i = g // 2
                    pe.wait_ge(xsA[i % 2], 16 * (i // 2 + 1))
                    if tt == 1:
                        pe.wait_ge(xsB[i % 2], 16 * (i // 2 + 1))
                    if g >= 8:
                        pe.wait_ge(dvsem, g - 7)  # psum bank g%8 evacuated
                    x5 = x5s[i % NSLOT]
                    ps = pss[g % 8]
                    for j, (dy, dx) in enumerate(TAPS_XY):
                        jy, py = divmod(dy + 2, 2)
                        jx, px = divmod(dx + 2, 2)
                        a0 = 8 * tt + jy
                        rhs = x5[
                            :, a0 : a0 + 8, py : py + 1, px : px + 1, jx : jx + 64
                        ]
                        mm = pe.matmul(
                            ps[:],
                            wtile[:, j * 128 : (j + 1) * 128],
                            rhs,
                            start=(j == 0),
                            stop=(j == len(TAPS_XY) - 1),
                        )
                        if j == len(TAPS_XY) - 1:
                            mm.then_inc(pesem, 1)

            @block.vector
            def _(dve):
                for g in range(NG):
                    s = g // 4
                    t = g % 4
                    M = OZ_PER[g // 8] * 16
                    if t == 0 and s >= 2:
                        dve.wait_ge(osem, 32 * (s - 1))  # stage slot s%2 free
                    dve.wait_ge(pesem, g + 1)
                    dve.tensor_copy(
                        out=stgs[s % 2][:M, t * 512 : (t + 1) * 512],
                        in_=pss[g % 8][:M],
                    ).then_inc(dvsem, 1)

    nc.compile()
    return nc


def _get_module():
    global _MODULE
    if _MODULE is None:
        _MODULE = _build_module()
    return _MODULE


def _band_weights(w5):
    """wc[k=(z*8+ic), j*128 + ozs*16 + oc] block-banded weights (ozs 6,7 pad)."""
    wc = np.zeros((128, 13, 8, 16), np.float32)
    for j, (dy, dx) in enumerate(TAPS_XY):
        for dzi in range(5):
            dz = dzi - 2
            if dz * dz + dy * dy + dx * dx > 4:
                continue
            blk = w5[:, :, dzi, dy + 2, dx + 2].T  # [ic, oc]
            for ozs in range(6):
                z = 2 * ozs + dzi
                wc[z * 8 : (z + 1) * 8, j, ozs, :] = blk
    return np.ascontiguousarray(wc.reshape(128, 13 * 128))


def _shard_core_input(x, b, gz):
    """Per-core padded input as 12 y-halved sub-units [128, 36*132]."""
    xp = np.zeros((IC, 40, 132, 132), BF16)
    z_lo = 32 * gz - 2
    src_lo, src_hi = max(0, z_lo), min(128, z_lo + 40)
    xp[:, src_lo - z_lo : src_hi - z_lo, 2:130, 2:130] = x[b, :, src_lo:src_hi, :, :]
    units = np.empty((12, 128, SUB_FREE), BF16)
    for c in range(3):
        for h in range(2):
            for q in range(2):
                u = xp[:, 12 * c : 12 * c + 16, 64 * h + 32 * q : 64 * h + 32 * q + 36, :]
                # de-interleave phases: free = (yo 18, yp 2, px 2, xe 66)
                u = u.reshape(IC, 16, 36, 66, 2).transpose(0, 1, 2, 4, 3)
                u = u.reshape(IC, 16, 18, 2, 2, 66)
                units[c * 4 + h * 2 + q] = u.transpose(1, 0, 2, 3, 4, 5).reshape(
                    128, SUB_FREE
                )
    return units


def kernel(x, weight, bias, psi_local):
    global LAST_RESULT
    from concourse.bass_utils import run_bass_kernel_spmd

    x = np.asarray(x, np.float32)
    weight = np.asarray(weight, np.float32)
    bias = np.asarray(bias, np.float32)
    psi_local = np.asarray(psi_local, np.float32)

    w5 = np.einsum("ogk,kzyx->ogzyx", weight, psi_local).astype(np.float32)
    wc = _band_weights(w5).astype(BF16)

    in_maps = []
    for core in range(N_CORES):
        b, gz = divmod(core, 4)
        in_maps.append({"xc": _shard_core_input(x, b, gz), "wc": wc})

    nc = _get_module()
    trace = bool(int(os.environ.get("KERNEL_TRACE", "0")))
    res = run_bass_kernel_spmd(
        nc, in_maps, core_ids=list(range(N_CORES)), trace=trace
    )
    LAST_RESULT = res

    out = np.empty((2, OC, 64, 64, 64), np.float32)
    for core in range(N_CORES):
        b, gz = divmod(core, 4)
        out[b, :, 16 * gz : 16 * gz + 16] = res.results[core]["out"].transpose(
            1, 0, 2, 3
        )
    out += bias[None, :, None, None, None]
    return out
